# revision 9
# baseline (speedup 1.0000x reference)
"""Trainium2 Bass kernel for MultiHeadLinearSelfAttention (linear attention +
LePE depthwise conv + projections), SPMD data-parallel over batch on 8 cores.

Algorithm (per image, channel-major [256, 4096] unless noted):
  qkv = W x + b (1x1 conv);  q/k = elu(.)+1;  per-head kv = k^T v, ksum = k^T 1
  attn = (q @ kv_blockdiag) * bcast(1/(q . ksum));  lepe = depthwise3x3(v)
  y = Wo (attn + lepe) + b_total

Device mapping:
  - q produced channel-major (x as moving operand), k/v pixel-major (x as
    stationary operand) so kv/ksum contract over pixels with zero transposes.
  - elu(z)+1 decomposed exactly as max(z,0) + min(e^z, 1):
      ACT: E=exp(z);  GPSIMD: G=min(E,1);  DVE: S=(z max 0) add G.
  - biases injected with K=1 rank-1 matmuls into PSUM.
  - normalization folded into q: recip(den) broadcast across head channels by
    a tiny 0/1-mask matmul, multiplied into S on DVE; numerator PSUM then
    directly accumulates the 9 LePE diagonal-matmul taps (per-channel weights
    on the diagonal of the stationary operand; image edges handled by 63-wide
    strided access patterns into a zero-padded v image).
  - final projection accumulates in PSUM and DMAs straight to HBM.
"""

import sys

for _p in ("/opt/trn_rl_repo",):
    if _p not in sys.path:
        sys.path.insert(0, _p)

import numpy as np
import ml_dtypes

import concourse.bass as bass
import concourse.bacc as bacc
import concourse.mybir as mybir
import concourse.tile as tile
from concourse.bass_utils import run_bass_kernel_spmd

BF16 = mybir.dt.bfloat16
F32 = mybir.dt.float32
AF = mybir.ActivationFunctionType
ALU = mybir.AluOpType

N_CORES = 8
IMG = 2            # images per core (B=16)
C = 256
N = 4096           # pixels (64x64)
W = 64
G = 2              # channel groups of 128
TAPS = [(ty - 1, tx - 1) for ty in range(3) for tx in range(3)]

_CACHE = {}


def build_program():
    nc = bacc.Bacc(
        "TRN2", target_bir_lowering=False, debug=False,
        enable_asserts=False, num_devices=N_CORES,
    )
    x_d = nc.dram_tensor("x", [IMG, C, N], BF16, kind="ExternalInput").ap()
    wqT_d = nc.dram_tensor("wqT", [C, 256], BF16, kind="ExternalInput").ap()
    wkvT_d = nc.dram_tensor("wkvT", [C, 512], BF16, kind="ExternalInput").ap()
    woT_d = nc.dram_tensor("woT", [C, 256], BF16, kind="ExternalInput").ap()
    bqr_d = nc.dram_tensor("bqr", [1, 256], BF16, kind="ExternalInput").ap()
    bkvr_d = nc.dram_tensor("bkvr", [1, 512], BF16, kind="ExternalInput").ap()
    btot_d = nc.dram_tensor("btotr", [1, 256], BF16, kind="ExternalInput").ap()
    bvc_d = nc.dram_tensor("bvc", [C, 1], F32, kind="ExternalInput").ap()
    lepe_d = nc.dram_tensor("lepe", [G, 9, 128, 128], BF16, kind="ExternalInput").ap()
    em_d = nc.dram_tensor("emask", [G, 8, 128], BF16, kind="ExternalInput").ap()
    y_d = nc.dram_tensor("y", [IMG, C, N], F32, kind="ExternalOutput").ap()

    with tile.TileContext(nc) as tc:
        with (
            tc.tile_pool(name="const", bufs=1) as const,
            tc.tile_pool(name="sb", bufs=1) as sb,
            tc.tile_pool(name="ps", bufs=1, space=bass.MemorySpace.PSUM) as ps,
        ):
            # ---- constants ----
            wq, wkv, wo, bvc, em, lep = [], [], [], [], [], []
            for g in range(G):
                t = const.tile([128, 256], BF16, name=f"wq{g}", tag=f"wq{g}")
                nc.sync.dma_start(t[:], wqT_d[g * 128:(g + 1) * 128, :])
                wq.append(t)
                t = const.tile([128, 512], BF16, name=f"wkv{g}", tag=f"wkv{g}")
                nc.sync.dma_start(t[:], wkvT_d[g * 128:(g + 1) * 128, :])
                wkv.append(t)
                t = const.tile([128, 256], BF16, name=f"wo{g}", tag=f"wo{g}")
                nc.sync.dma_start(t[:], woT_d[g * 128:(g + 1) * 128, :])
                wo.append(t)
                t = const.tile([128, 1], F32, name=f"bvc{g}", tag=f"bvc{g}")
                nc.sync.dma_start(t[:], bvc_d[g * 128:(g + 1) * 128, :])
                bvc.append(t)
                t = const.tile([8, 128], BF16, name=f"em{g}", tag=f"em{g}")
                nc.sync.dma_start(t[:], em_d[g])
                em.append(t)
                row = []
                for ti in range(9):
                    t = const.tile([128, 128], BF16, name=f"lep{g}_{ti}", tag=f"lep{g}_{ti}")
                    nc.sync.dma_start(t[:], lepe_d[g, ti])
                    row.append(t)
                lep.append(row)
            bqr = const.tile([1, 256], BF16, name="bqr", tag="bqr")
            nc.sync.dma_start(bqr[:], bqr_d[:])
            bkvr = const.tile([1, 512], BF16, name="bkvr", tag="bkvr")
            nc.sync.dma_start(bkvr[:], bkvr_d[:])
            btot = const.tile([1, 256], BF16, name="btot", tag="btot")
            nc.sync.dma_start(btot[:], btot_d[:])
            ones512 = const.tile([1, 512], BF16, name="ones512", tag="ones512")
            nc.gpsimd.memset(ones512[:], 1.0)
            ones128 = const.tile([1, 128], BF16, name="ones128", tag="ones128")
            nc.gpsimd.memset(ones128[:], 1.0)

            for img in range(IMG):
                # ---- load x ----
                xs = []
                for g in range(G):
                    t = sb.tile([128, N], BF16, name=f"x{img}_{g}", tag=f"x{g}", bufs=2)
                    nc.sync.dma_start(t[:], x_d[img, g * 128:(g + 1) * 128, :])
                    xs.append(t)
                # padded v image: 66x66 grid, data at rows 1..64, cols 1..64
                vpad = []
                for g in range(G):
                    t = sb.tile([128, 66 * 66], BF16, name=f"vp{img}_{g}",
                                tag=f"vp{g}", bufs=2)
                    nc.gpsimd.memset(t[:], 0.0)
                    vpad.append(t)

                # ---- phase A: k/v pixel-major, kv+ksum accumulation ----
                kvp = [ps.tile([128, 129], F32, name=f"kv{img}_{g}", tag=f"kv{g}")
                       for g in range(G)]
                for nt in range(32):
                    kvpix = ps.tile([128, 512], F32, name=f"kvpix{img}_{nt}",
                                    tag="big", bufs=2)
                    nc.tensor.matmul(kvpix[:], xs[0][:, nt * 128:(nt + 1) * 128],
                                     wkv[0][:], start=True, stop=False)
                    nc.tensor.matmul(kvpix[:], xs[1][:, nt * 128:(nt + 1) * 128],
                                     wkv[1][:], start=False, stop=False)
                    nc.tensor.matmul(kvpix[:], ones128[:], bkvr[:],
                                     start=False, stop=True)
                    ek = sb.tile([128, 256], F32, name=f"ek{img}_{nt}", tag="ek", bufs=3)
                    nc.scalar.activation(ek[:], kvpix[:, 0:256], AF.Exp)
                    gk = sb.tile([128, 256], F32, name=f"gk{img}_{nt}", tag="gk", bufs=3)
                    nc.gpsimd.tensor_scalar_min(gk[:], ek[:], 1.0)
                    kh = sb.tile([128, 256], BF16, name=f"kh{img}_{nt}", tag="kh", bufs=3)
                    nc.vector.scalar_tensor_tensor(kh[:], kvpix[:, 0:256], 0.0, gk[:],
                                                   ALU.max, ALU.add)
                    vsb = sb.tile([128, 258], BF16, name=f"vsb{img}_{nt}", tag="vsb", bufs=3)
                    nc.scalar.copy(vsb[:, 0:128], kvpix[:, 256:384])
                    nc.scalar.copy(vsb[:, 129:257], kvpix[:, 384:512])
                    nc.gpsimd.memset(vsb[:, 128:129], 1.0)
                    nc.gpsimd.memset(vsb[:, 257:258], 1.0)
                    for g in range(G):
                        nc.tensor.matmul(kvp[g][:], kh[:, g * 128:(g + 1) * 128],
                                         vsb[:, g * 129:(g + 1) * 129],
                                         start=(nt == 0), stop=(nt == 31))

                # ---- phase A2: v channel-major into padded image ----
                for g in range(G):
                    for nt in range(8):
                        vch = ps.tile([128, 512], F32, name=f"vch{img}_{g}_{nt}",
                                      tag="big", bufs=2)
                        nc.tensor.matmul(vch[:],
                                         wkv[0][:, 256 + g * 128:256 + (g + 1) * 128],
                                         xs[0][:, nt * 512:(nt + 1) * 512],
                                         start=True, stop=False)
                        nc.tensor.matmul(vch[:],
                                         wkv[1][:, 256 + g * 128:256 + (g + 1) * 128],
                                         xs[1][:, nt * 512:(nt + 1) * 512],
                                         start=False, stop=True)
                        vp3 = vpad[g][:].rearrange("p (r x) -> p r x", x=66)
                        nc.scalar.activation(vp3[:, 8 * nt + 1:8 * nt + 9, 1:65],
                                             vch[:], AF.Identity, bias=bvc[g][:])

                # ---- phase B: kv evacuation to blockdiag + KS ----
                bd, KS = [], []
                for g in range(G):
                    b = sb.tile([128, 128], BF16, name=f"bd{img}_{g}", tag=f"bd{g}", bufs=2)
                    nc.gpsimd.memset(b[:], 0.0)
                    ks = sb.tile([128, 8], BF16, name=f"KS{img}_{g}", tag=f"KS{g}", bufs=2)
                    nc.gpsimd.memset(ks[:], 0.0)
                    for h in range(4):
                        sl = slice(h * 32, (h + 1) * 32)
                        nc.scalar.copy(b[sl, sl], kvp[g][sl, sl])
                        nc.scalar.copy(ks[sl, g * 4 + h:g * 4 + h + 1],
                                       kvp[g][sl, 128:129])
                    bd.append(b)
                    KS.append(ks)

                # ---- phase C: q, normalization, attention+lepe, projection ----
                for nt in range(8):
                    Sq = []
                    for o in range(G):
                        qp = ps.tile([128, 512], F32, name=f"qp{img}_{o}_{nt}",
                                     tag="big", bufs=2)
                        nc.tensor.matmul(qp[:], wq[0][:, o * 128:(o + 1) * 128],
                                         xs[0][:, nt * 512:(nt + 1) * 512],
                                         start=True, stop=False)
                        nc.tensor.matmul(qp[:], wq[1][:, o * 128:(o + 1) * 128],
                                         xs[1][:, nt * 512:(nt + 1) * 512],
                                         start=False, stop=False)
                        nc.tensor.matmul(qp[:], bqr[:, o * 128:(o + 1) * 128],
                                         ones512[:], start=False, stop=True)
                        eq = sb.tile([128, 512], F32, name=f"eq{img}_{o}_{nt}",
                                     tag="eq", bufs=2)
                        nc.scalar.activation(eq[:], qp[:], AF.Exp)
                        gq = sb.tile([128, 512], F32, name=f"gq{img}_{o}_{nt}",
                                     tag="gq", bufs=2)
                        nc.gpsimd.tensor_scalar_min(gq[:], eq[:], 1.0)
                        S = sb.tile([128, 512], BF16, name=f"S{img}_{o}_{nt}",
                                    tag=f"S{o}", bufs=2)
                        nc.vector.scalar_tensor_tensor(S[:], qp[:], 0.0, gq[:],
                                                       ALU.max, ALU.add)
                        Sq.append(S)
                    den = ps.tile([8, 512], F32, name=f"den{img}_{nt}", tag="denbc")
                    nc.tensor.matmul(den[:], KS[0][:], Sq[0][:],
                                     start=True, stop=False)
                    nc.tensor.matmul(den[:], KS[1][:], Sq[1][:],
                                     start=False, stop=True)
                    rc = sb.tile([8, 512], BF16, name=f"rc{img}_{nt}", tag="rc", bufs=2)
                    with nc.allow_low_precision(reason="recip feeds bf16 matmul; den is O(1e3), bf16 rel err 2e-3 acceptable"):
                        nc.vector.reciprocal(rc[:], den[:])
                    rats = []
                    for g in range(G):
                        bc = ps.tile([128, 512], F32, name=f"bc{img}_{g}_{nt}",
                                     tag="denbc")
                        nc.tensor.matmul(bc[:], em[g][:], rc[:], start=True, stop=True)
                        qh = sb.tile([128, 512], BF16, name=f"qh{img}_{g}_{nt}",
                                     tag=f"qh{g}", bufs=2)
                        nc.vector.tensor_mul(qh[:], Sq[g][:], bc[:])
                        at = ps.tile([128, 512], F32, name=f"at{img}_{g}_{nt}",
                                     tag="attn", bufs=2)
                        nc.tensor.matmul(at[:], bd[g][:], qh[:], start=True, stop=False)
                        vp3 = vpad[g][:].rearrange("p (r x) -> p r x", x=66)
                        for ti, (dy, dx) in enumerate(TAPS):
                            last = ti == len(TAPS) - 1
                            inap = vp3[:, 8 * nt + dy + 1:8 * nt + dy + 9,
                                       1 + dx:1 + dx + 64]
                            nc.tensor.matmul(at[:], lep[g][ti][:], inap,
                                             start=False, stop=last)
                        rat = sb.tile([128, 512], BF16, name=f"rat{img}_{g}_{nt}",
                                      tag=f"rat{g}", bufs=2)
                        nc.scalar.copy(rat[:], at[:])
                        rats.append(rat)
                    for o in range(G):
                        op_ = ps.tile([128, 512], F32, name=f"op{img}_{o}_{nt}",
                                      tag="outp")
                        nc.tensor.matmul(op_[:], wo[0][:, o * 128:(o + 1) * 128],
                                         rats[0][:], start=True, stop=False)
                        nc.tensor.matmul(op_[:], wo[1][:, o * 128:(o + 1) * 128],
                                         rats[1][:], start=False, stop=False)
                        nc.tensor.matmul(op_[:], btot[:, o * 128:(o + 1) * 128],
                                         ones512[:], start=False, stop=True)
                        ot = sb.tile([128, 512], F32, name=f"ot{img}_{o}_{nt}",
                                     tag=f"ot{o}", bufs=2)
                        if o == 0:
                            nc.scalar.copy(ot[:], op_[:])
                        else:
                            nc.vector.tensor_copy(ot[:], op_[:])
                        nc.sync.dma_start(
                            y_d[img, o * 128:(o + 1) * 128, nt * 512:(nt + 1) * 512],
                            ot[:])

    nc.compile()
    return nc


def _prep_inputs(x, qkv_w, qkv_b, lepe_w, lepe_b, out_w, out_b):
    bf = ml_dtypes.bfloat16
    x = np.ascontiguousarray(np.asarray(x, np.float32)).reshape(16, C, N)
    qkv_w = np.asarray(qkv_w, np.float32)
    qkv_b = np.asarray(qkv_b, np.float32)
    lepe_w = np.asarray(lepe_w, np.float32)
    lepe_b = np.asarray(lepe_b, np.float32)
    out_w = np.asarray(out_w, np.float32)
    out_b = np.asarray(out_b, np.float32)

    wqT = np.ascontiguousarray(qkv_w[0:256].T).astype(bf)
    wkvT = np.ascontiguousarray(
        np.concatenate([qkv_w[256:512].T, qkv_w[512:768].T], axis=1)).astype(bf)
    woT = np.ascontiguousarray(out_w.T).astype(bf)
    bqr = qkv_b[0:256].reshape(1, 256).astype(bf)
    bkvr = qkv_b[256:768].reshape(1, 512).astype(bf)
    btot = (out_b + out_w @ lepe_b).reshape(1, 256).astype(bf)
    bvc = qkv_b[512:768].reshape(C, 1).astype(np.float32)
    lepe = np.zeros((G, 9, 128, 128), np.float32)
    for g in range(G):
        for ti, (dy, dx) in enumerate(TAPS):
            wcol = lepe_w[g * 128:(g + 1) * 128, 0, dy + 1, dx + 1]
            np.fill_diagonal(lepe[g, ti], wcol)
    lepe = lepe.astype(bf)
    emask = np.zeros((G, 8, 128), np.float32)
    for g in range(G):
        for e in range(128):
            emask[g, (g * 128 + e) // 32, e] = 1.0
    emask = emask.astype(bf)

    shared = dict(wqT=wqT, wkvT=wkvT, woT=woT, bqr=bqr, bkvr=bkvr,
                  btotr=btot, bvc=bvc, lepe=lepe, emask=emask)
    in_maps = []
    for c in range(N_CORES):
        m = dict(shared)
        m["x"] = x[c * IMG:(c + 1) * IMG].astype(bf)
        in_maps.append(m)
    return in_maps


def kernel(x, qkv_w, qkv_b, lepe_w, lepe_b, out_w, out_b):
    if "nc" not in _CACHE:
        _CACHE["nc"] = build_program()
    nc = _CACHE["nc"]
    in_maps = _prep_inputs(x, qkv_w, qkv_b, lepe_w, lepe_b, out_w, out_b)
    res = run_bass_kernel_spmd(nc, in_maps, core_ids=list(range(N_CORES)))
    out = np.concatenate([np.asarray(r["y"], np.float32) for r in res.results])
    return out.reshape(16, C, 64, 64)


if __name__ == "__main__":
    build_program()
    print("build OK")


# revision 36
# speedup vs baseline: 65.2333x; 65.2333x over previous
"""Trainium2 Bass kernel for MultiHeadLinearSelfAttention (linear attention +
LePE depthwise conv + projections), SPMD data-parallel over batch on 8 cores.

Algorithm (per image, channel-major [256, 4096] unless noted):
  qkv = W x + b (1x1 conv);  q/k = elu(.)+1;  per-head kv = k^T v, ksum = k^T 1
  attn = (q @ kv_blockdiag) * bcast(1/(q . ksum));  lepe = depthwise3x3(v)
  y = Wo (attn + lepe) + b_total

Device mapping highlights:
  - q produced channel-major (x as moving operand), k/v pixel-major (x as
    stationary operand) so kv/ksum contract over pixels with zero transposes.
  - elu(z)+1 decomposed exactly as max(z,0) + min(e^z, 1):
      ACT: E=exp(z);  GPSIMD: G=min(E,1);  DVE: S=(z max 0) add G.
  - k/q biases via K=1 rank-1 matmuls into PSUM; v bias is NOT applied in the
    attention path: since sum_d qhat*ksum == 1 per head, its effect is a
    constant per-channel shift folded into the output bias on the host. v bias
    IS applied (per-partition ACT bias) on the channel-major copy feeding
    LePE, whose edge effects need the true v.
  - normalization folded into q: recip(den) broadcast across head channels by
    a tiny 0/1-mask matmul, multiplied into S on DVE; the numerator PSUM then
    directly accumulates the 9 LePE diagonal-matmul taps (per-channel weights
    on the diagonal of the stationary operand; image edges via a 66x66
    zero-ringed padded v image and 3D access patterns).
  - final projection accumulates in PSUM; output bias rides the evacuation
    (ACT Identity-bias / DVE tensor_scalar_add), then DMA to HBM.
"""

import os
import sys

for _p in ("/opt/trn_rl_repo",):
    if _p not in sys.path:
        sys.path.insert(0, _p)

import numpy as np
import ml_dtypes

import concourse.bass as bass
import concourse.bacc as bacc
import concourse.mybir as mybir
import concourse.tile as tile
from concourse.bass_utils import run_bass_kernel_spmd

BF16 = mybir.dt.bfloat16
F32 = mybir.dt.float32
AF = mybir.ActivationFunctionType
ALU = mybir.AluOpType

N_CORES = 8
IMG = 2            # images per core (B=16)
C = 256
N = 4096           # pixels (64x64)
G = 2              # channel groups of 128
TAPS = [(ty - 1, tx - 1) for ty in range(3) for tx in range(3)]

REPS = int(os.environ.get("BK_REPS", "1"))
SKIP = set(filter(None, os.environ.get("BK_SKIP", "").split(",")))

_CACHE = {}


def build_program():
    nc = bacc.Bacc(
        "TRN2", target_bir_lowering=False, debug=False,
        enable_asserts=False, num_devices=N_CORES,
    )
    x_d = nc.dram_tensor("x", [IMG, C, N], BF16, kind="ExternalInput").ap()
    wpack_d = nc.dram_tensor("wpack", [G, 128, 2176], BF16, kind="ExternalInput").ap()
    brow_d = nc.dram_tensor("brow", [1, 512], BF16, kind="ExternalInput").ap()
    bcol_d = nc.dram_tensor("bcol", [C, 2], F32, kind="ExternalInput").ap()
    em_d = nc.dram_tensor("emask", [8, 256], BF16, kind="ExternalInput").ap()
    y_d = nc.dram_tensor("y", [IMG, C, N], F32, kind="ExternalOutput").ap()

    with tile.TileContext(nc) as tc:
        with (
            tc.tile_pool(name="const", bufs=1) as const,
            tc.tile_pool(name="sb", bufs=1) as sb,
            tc.tile_pool(name="ps", bufs=1, space=bass.MemorySpace.PSUM) as ps,
        ):
            # ---------------- constants (few big DMAs) ----------------
            wq, wkv, wo, bvc, btc, em, lep = [], [], [], [], [], [], []
            wp = []
            for g in range(G):
                t = const.tile([128, 2176], BF16, name=f"wp{g}", tag=f"wp{g}")
                nc.scalar.dma_start(t[:], wpack_d[g])
                wp.append(t)
                wq.append(t[:, 0:256])
                wkv.append(t[:, 256:768])
                wo.append(t[:, 768:1024])
                lep.append([t[:, 1024 + ti * 128:1024 + (ti + 1) * 128]
                            for ti in range(9)])
            emt = const.tile([8, 256], BF16, name="emt", tag="emt")
            nc.sync.dma_start(emt[:], em_d[:])
            em = [emt[:, 0:128], emt[:, 128:256]]
            brow = const.tile([1, 512], BF16, name="brow", tag="brow")
            nc.sync.dma_start(brow[:], brow_d[:])
            bqr, bkr = brow[:, 0:256], brow[:, 256:512]
            bct = []
            for g in range(G):
                t = const.tile([128, 2], F32, name=f"bct{g}", tag=f"bct{g}")
                nc.sync.dma_start(t[:], bcol_d[g * 128:(g + 1) * 128, :])
                bct.append(t)
            bvc = [bct[g][:, 0:1] for g in range(G)]
            btc = [bct[g][:, 1:2] for g in range(G)]
            ones512 = const.tile([1, 512], BF16, name="ones512", tag="ones512")
            nc.gpsimd.memset(ones512[:], 1.0)
            ones128 = const.tile([1, 128], BF16, name="ones128", tag="ones128")
            nc.gpsimd.memset(ones128[:], 1.0)

            st = {}   # per-image state

            # ---------------- phases ----------------
            def load_x(u, img0):
                xs = []
                for g in range(G):
                    t = sb.tile([128, N], BF16, name=f"x{u}_{g}", tag=f"x{g}", bufs=2)
                    nc.gpsimd.dma_start(t[:], x_d[img0, g * 128:(g + 1) * 128, :])
                    xs.append(t)
                st[u] = {"xs": xs}

            def phase_a(u):
                xs = st[u]["xs"]
                kvp = [ps.tile([128, 129], F32, name=f"kv{u}_{g}",
                                tag=("kv0" if g == 0 else "kv1"),
                                bufs=1)
                       for g in range(G)]
                st[u]["kvp"] = kvp
                vpad = []
                for g in range(G):
                    t = sb.tile([128, 66 * 66], BF16, name=f"vp{u}_{g}",
                                tag=f"vp{g}", bufs=2)
                    nc.gpsimd.memset(t[:], 0.0)
                    vpad.append(t)
                st[u]["vpad"] = vpad
                for nt in range(32):
                    a2_iter(u, nt)
                    kvpix = ps.tile([128, 512], F32, name=f"kvpix{u}_{nt}",
                                    tag="big", bufs=4)
                    nc.tensor.matmul(kvpix[:], xs[0][:, nt * 128:(nt + 1) * 128],
                                     wkv[0], start=True, stop=False)
                    nc.tensor.matmul(kvpix[:], xs[1][:, nt * 128:(nt + 1) * 128],
                                     wkv[1], start=False, stop=False)
                    nc.tensor.matmul(kvpix[:, 0:256], ones128[:], bkr[:],
                                     start=False, stop=True)
                    kh = sb.tile([128, 256], BF16, name=f"kh{u}_{nt}", tag="kh", bufs=5)
                    if "elu" in SKIP:
                        nc.scalar.copy(kh[:], kvpix[:, 0:256])
                    else:
                        ek = sb.tile([128, 256], BF16, name=f"ek{u}_{nt}", tag="ek", bufs=5)
                        nc.scalar.activation(ek[:], kvpix[:, 0:256], AF.Exp)
                        gk = sb.tile([128, 256], BF16, name=f"gk{u}_{nt}", tag="gk", bufs=5)
                        nc.gpsimd.tensor_scalar_min(gk[:], ek[:], 1.0)
                        nc.vector.scalar_tensor_tensor(kh[:], kvpix[:, 0:256], 0.0,
                                                       gk[:], ALU.max, ALU.add)
                    vsb = sb.tile([128, 258], BF16, name=f"vsb{u}_{nt}", tag="vsb", bufs=5)
                    nc.vector.tensor_copy(vsb[:, 1:257], kvpix[:, 256:512])
                    nc.gpsimd.memset(vsb[:, 0:1], 1.0)
                    nc.gpsimd.memset(vsb[:, 257:258], 1.0)
                    # g0 rhs = [1 | v0] -> psum col 0 = ksum; g1 rhs = [v1 | 1] -> col 128
                    nc.tensor.matmul(kvp[0][:], kh[:, 0:128], vsb[:, 0:129],
                                     start=(nt == 0), stop=(nt == 31))
                    nc.tensor.matmul(kvp[1][:], kh[:, 128:256], vsb[:, 129:258],
                                     start=(nt == 0), stop=(nt == 31))

            def a2_iter(u, nt32):
                # one (g, nt8) v_ch tile per two phase-A iterations
                if nt32 % 2 != 0:
                    return
                j = nt32 // 2
                g, nt = j // 8, j % 8
                xs, vpad = st[u]["xs"], st[u]["vpad"]
                vch = ps.tile([128, 512], F32, name=f"vch{u}_{g}_{nt}",
                              tag="big", bufs=4)
                nc.tensor.matmul(vch[:],
                                 wkv[0][:, 256 + g * 128:256 + (g + 1) * 128],
                                 xs[0][:, nt * 512:(nt + 1) * 512],
                                 start=True, stop=False)
                nc.tensor.matmul(vch[:],
                                 wkv[1][:, 256 + g * 128:256 + (g + 1) * 128],
                                 xs[1][:, nt * 512:(nt + 1) * 512],
                                 start=False, stop=True)
                vp3 = vpad[g][:].rearrange("p (r x) -> p r x", x=66)
                nc.scalar.activation(vp3[:, 8 * nt + 1:8 * nt + 9, 1:65],
                                     vch[:], AF.Identity, bias=bvc[g])

            def phase_b(u):
                kvp = st[u]["kvp"]
                bd, KS = [], []
                for g in range(G):
                    kvoff = 1 if g == 0 else 0
                    ksoff = 0 if g == 0 else 128
                    b = sb.tile([128, 128], BF16, name=f"bd{u}_{g}", tag=f"bd{g}", bufs=2)
                    nc.gpsimd.memset(b[:], 0.0)
                    ks = sb.tile([128, 8], BF16, name=f"KS{u}_{g}", tag=f"KS{g}", bufs=2)
                    nc.gpsimd.memset(ks[:], 0.0)
                    for h in range(4):
                        sl = slice(h * 32, (h + 1) * 32)
                        nc.scalar.copy(b[sl, sl],
                                       kvp[g][sl, kvoff + h * 32:kvoff + (h + 1) * 32])
                        nc.scalar.copy(ks[sl, g * 4 + h:g * 4 + h + 1],
                                       kvp[g][sl, ksoff:ksoff + 1])
                    bd.append(b)
                    KS.append(ks)
                st[u]["bd"], st[u]["KS"] = bd, KS

            def c_iter(u, img0, nt):
                    xs, vpad = st[u]["xs"], st[u]["vpad"]
                    bd, KS = st[u]["bd"], st[u]["KS"]
                    Sq = []
                    for o in range(G):
                        qp = ps.tile([128, 512], F32, name=f"qp{u}_{o}_{nt}",
                                     tag="big", bufs=4)
                        nc.tensor.matmul(qp[:], wq[0][:, o * 128:(o + 1) * 128],
                                         xs[0][:, nt * 512:(nt + 1) * 512],
                                         start=True, stop=False)
                        nc.tensor.matmul(qp[:], wq[1][:, o * 128:(o + 1) * 128],
                                         xs[1][:, nt * 512:(nt + 1) * 512],
                                         start=False, stop=False)
                        nc.tensor.matmul(qp[:], bqr[:, o * 128:(o + 1) * 128],
                                         ones512[:], start=False, stop=True)
                        S = sb.tile([128, 512], BF16, name=f"S{u}_{o}_{nt}",
                                    tag=f"S{o}", bufs=4)
                        if "elu" in SKIP:
                            nc.scalar.copy(S[:], qp[:])
                        else:
                            eq = sb.tile([128, 512], BF16, name=f"eq{u}_{o}_{nt}",
                                         tag="eq", bufs=4)
                            nc.scalar.activation(eq[:], qp[:], AF.Exp)
                            gq = sb.tile([128, 512], BF16, name=f"gq{u}_{o}_{nt}",
                                         tag="gq", bufs=4)
                            nc.gpsimd.tensor_scalar_min(gq[:], eq[:], 1.0)
                            nc.vector.scalar_tensor_tensor(S[:], qp[:], 0.0, gq[:],
                                                           ALU.max, ALU.add)
                        Sq.append(S)
                    den = ps.tile([8, 512], F32, name=f"den{u}_{nt}", tag="kv0", bufs=1)
                    nc.tensor.matmul(den[:], KS[0][:], Sq[0][:], start=True, stop=False)
                    nc.tensor.matmul(den[:], KS[1][:], Sq[1][:], start=False, stop=True)
                    rc = sb.tile([8, 512], BF16, name=f"rc{u}_{nt}", tag="rc", bufs=3)
                    with nc.allow_low_precision(reason="recip feeds bf16 matmul"):
                        nc.vector.reciprocal(rc[:], den[:])
                    rats = []
                    for g in range(G):
                        bc = ps.tile([128, 512], F32, name=f"bc{u}_{g}_{nt}",
                                     tag="kv1", bufs=1)
                        nc.tensor.matmul(bc[:], em[g], rc[:], start=True, stop=True)
                        qh = sb.tile([128, 512], BF16, name=f"qh{u}_{g}_{nt}",
                                     tag=f"qh{g}", bufs=3)
                        nc.vector.tensor_mul(qh[:], Sq[g][:], bc[:])
                        at = ps.tile([128, 512], F32, name=f"at{u}_{g}_{nt}",
                                     tag="attn", bufs=2)
                        nc.tensor.matmul(at[:], bd[g][:], qh[:], start=True,
                                         stop=("lepe" in SKIP))
                        if "lepe" not in SKIP:
                            vp3 = vpad[g][:].rearrange("p (r x) -> p r x", x=66)
                            for ti, (dy, dx) in enumerate(TAPS):
                                last = ti == len(TAPS) - 1
                                inap = vp3[:, 8 * nt + dy + 1:8 * nt + dy + 9,
                                           1 + dx:1 + dx + 64]
                                nc.tensor.matmul(at[:], lep[g][ti], inap,
                                                 start=False, stop=last)
                        rat = sb.tile([128, 512], BF16, name=f"rat{u}_{g}_{nt}",
                                      tag=f"rat{g}", bufs=3)
                        nc.scalar.copy(rat[:], at[:])
                        rats.append(rat)
                    for o in range(G):
                        op_ = ps.tile([128, 512], F32, name=f"op{u}_{o}_{nt}",
                                      tag="attn", bufs=2)
                        nc.tensor.matmul(op_[:], wo[0][:, o * 128:(o + 1) * 128],
                                         rats[0][:], start=True, stop=False)
                        nc.tensor.matmul(op_[:], wo[1][:, o * 128:(o + 1) * 128],
                                         rats[1][:], start=False, stop=True)
                        ot = sb.tile([128, 512], F32, name=f"ot{u}_{o}_{nt}",
                                     tag=f"ot{o}", bufs=2)
                        if o == 0:
                            nc.scalar.activation(ot[:], op_[:], AF.Identity,
                                                 bias=btc[o])
                        else:
                            nc.vector.tensor_scalar_add(ot[:], op_[:], btc[o])
                        nc.sync.dma_start(
                            y_d[img0, o * 128:(o + 1) * 128, nt * 512:(nt + 1) * 512],
                            ot[:])

            # ---------------- schedule: interleave the two images ----------------
            for rep in range(REPS):
                us = [rep * IMG + i for i in range(IMG)]
                for i, u in enumerate(us):
                    load_x(u, i)
                for u in us:
                    phase_a(u)
                for u in us:
                    phase_b(u)
                for nt in range(8):
                    for i, u in enumerate(us):
                        c_iter(u, i, nt)

    nc.compile()
    return nc


def _prep_inputs(x, qkv_w, qkv_b, lepe_w, lepe_b, out_w, out_b):
    bf = ml_dtypes.bfloat16
    x = np.ascontiguousarray(np.asarray(x, np.float32)).reshape(16, C, N)
    qkv_w = np.asarray(qkv_w, np.float32)
    qkv_b = np.asarray(qkv_b, np.float32)
    lepe_w = np.asarray(lepe_w, np.float32)
    lepe_b = np.asarray(lepe_b, np.float32)
    out_w = np.asarray(out_w, np.float32)
    out_b = np.asarray(out_b, np.float32)

    wqT = qkv_w[0:256].T
    wkvT = np.concatenate([qkv_w[256:512].T, qkv_w[512:768].T], axis=1)
    woT = out_w.T
    lepe = np.zeros((G, 9, 128, 128), np.float32)
    for g in range(G):
        for ti, (dy, dx) in enumerate(TAPS):
            wcol = lepe_w[g * 128:(g + 1) * 128, 0, dy + 1, dx + 1]
            np.fill_diagonal(lepe[g, ti], wcol)
    wpack = np.zeros((G, 128, 2176), np.float32)
    for g in range(G):
        sl = slice(g * 128, (g + 1) * 128)
        wpack[g, :, 0:256] = wqT[sl]
        wpack[g, :, 256:768] = wkvT[sl]
        wpack[g, :, 768:1024] = woT[sl]
        wpack[g, :, 1024:2176] = lepe[g].transpose(1, 0, 2).reshape(128, 1152)
    wpack = wpack.astype(bf)
    brow = np.concatenate([qkv_b[0:256], qkv_b[256:512]]).reshape(1, 512).astype(bf)
    bv = qkv_b[512:768]
    # v-bias in the attention path collapses to a constant per-channel shift
    # (sum_d qhat*ksum == 1), folded into the output bias here.
    btotc = out_b + out_w @ lepe_b + out_w @ bv
    bcol = np.stack([bv, btotc], axis=1).astype(np.float32)
    emask = np.zeros((8, 256), np.float32)
    for e in range(256):
        emask[e // 32, (e // 128) * 128 + e % 128] = 1.0
    emask = emask.astype(bf)

    shared = dict(wpack=wpack, brow=brow, bcol=bcol, emask=emask)
    in_maps = []
    for c in range(N_CORES):
        m = dict(shared)
        m["x"] = x[c * IMG:(c + 1) * IMG].astype(bf)
        in_maps.append(m)
    return in_maps


def kernel(x, qkv_w, qkv_b, lepe_w, lepe_b, out_w, out_b):
    if "nc" not in _CACHE:
        _CACHE["nc"] = build_program()
    nc = _CACHE["nc"]
    in_maps = _prep_inputs(x, qkv_w, qkv_b, lepe_w, lepe_b, out_w, out_b)
    res = run_bass_kernel_spmd(nc, in_maps, core_ids=list(range(N_CORES)))
    out = np.concatenate([np.asarray(r["y"], np.float32) for r in res.results])
    return out.reshape(16, C, 64, 64)


if __name__ == "__main__":
    build_program()
    print("build OK")
